# revision 2
# baseline (speedup 1.0000x reference)
"""Differential attention kernel for 8 TRN2 NeuronCores.

Strategy: data-parallel over the 16384 tokens (2048/core).  Per core:
  - PE-transpose x and the four weight matrices into contraction-major
    (bf16) layout.
  - Four 2048x2048 projections run on the TensorEngine in bf16
    (fp32 PSUM accumulate).
  - The per-token 32x32 head attention (q/k gram, softmax, diff, @v,
    RMS norm) runs on the Vector/GpSimd/Scalar engines with tokens on
    partitions, overlapped with PE work via the Tile scheduler.
"""

import numpy as np

DIM = 2048
H = 16
D = 64
LAYER_NUM = 12
LAM_INIT = 0.8 - 0.6 * float(np.exp(-0.3 * LAYER_NUM))
SCALE = D ** -0.5
EPS = 1e-5

NCORES = 8
T = 2048                  # tokens per core
NT = T // 128             # 16 token tiles per core
P = 128
OC = 4                    # output chunks of 512 per projection
KT = DIM // P             # 16 contraction tiles

_PROGRAM_CACHE = {}


def _build(lam: float):
    import concourse.bass as bass
    import concourse.tile as tile
    from concourse import bacc
    from concourse import mybir
    from concourse.masks import make_identity

    f32 = mybir.dt.float32
    bf16 = mybir.dt.bfloat16
    Alu = mybir.AluOpType
    Act = mybir.ActivationFunctionType
    Axis = mybir.AxisListType

    nc = bacc.Bacc("TRN2", target_bir_lowering=False)

    x_d = nc.dram_tensor("x", [T, DIM], f32, kind="ExternalInput")
    wq_d = nc.dram_tensor("Wq", [DIM, DIM], f32, kind="ExternalInput")
    wk_d = nc.dram_tensor("Wk", [DIM, DIM], f32, kind="ExternalInput")
    wv_d = nc.dram_tensor("Wv", [DIM, DIM], f32, kind="ExternalInput")
    wo_d = nc.dram_tensor("Wo", [DIM, DIM], f32, kind="ExternalInput")
    bo_d = nc.dram_tensor("bo", [P, DIM], f32, kind="ExternalInput")
    y_d = nc.dram_tensor("y", [T, DIM], f32, kind="ExternalOutput")

    with tile.TileContext(nc) as tc:
        with (
            tc.tile_pool(name="persist", bufs=1) as persist,
            tc.tile_pool(name="wpool", bufs=1) as wpool,
            tc.tile_pool(name="temps", bufs=2) as temps,
            tc.tile_pool(name="cast", bufs=3) as cast_p,
            tc.tile_pool(name="psum_mm", bufs=3, space="PSUM") as psum_mm,
            tc.tile_pool(name="psum_tr", bufs=3, space="PSUM") as psum_tr,
            tc.tile_pool(name="dram", bufs=1, space="DRAM") as dram,
        ):
            ident = persist.tile([P, P], bf16)
            make_identity(nc, ident)

            eps_sb = persist.tile([P, 1], f32)
            nc.vector.memset(eps_sb, EPS)

            # bias pre-replicated host-side to [P, DIM]
            bo_sb = persist.tile([P, DIM], f32)
            nc.sync.dma_start(out=bo_sb, in_=bo_d[:, :])

            # ---- transpose helper: DRAM [r, c] f32 -> SBUF [128, c/128, r] bf16
            def load_transposed(src_d, dst_sb, scale=None):
                for r in range(KT):
                    wf = temps.tile([P, DIM], f32, tag="ldT_f32")
                    nc.sync.dma_start(out=wf, in_=src_d[r * P:(r + 1) * P, :])
                    wb = cast_p.tile([P, DIM], bf16, tag="ldT_bf")
                    if scale is None:
                        nc.vector.tensor_copy(out=wb, in_=wf)
                    else:
                        nc.vector.tensor_scalar_mul(wb, wf, scale)
                    for c in range(KT):
                        ps = psum_tr.tile([P, P], bf16, tag="trps")
                        nc.tensor.transpose(
                            ps, wb[:, c * P:(c + 1) * P], ident)
                        nc.vector.tensor_copy(
                            out=dst_sb[:, c, r * P:(r + 1) * P], in_=ps)

            # DRAM spill for q/k/v (bf16, token-major)
            q_spill = dram.tile([T, DIM], bf16)
            k_spill = dram.tile([T, DIM], bf16)
            v_spill = dram.tile([T, DIM], bf16)

            # ---- projection: y[t,o] = sum_i x[t,i] W[o,i]
            def project(xT, w_dram, sink, scale=None):
                wT = wpool.tile([P, KT, DIM], bf16, tag="wT")
                load_transposed(w_dram, wT, scale=scale)
                for tt in range(NT):
                    for oc in range(OC):
                        ps = psum_mm.tile([P, 512], f32, tag="mmps")
                        for kt in range(KT):
                            nc.tensor.matmul(
                                ps,
                                lhsT=xT[:, kt, tt * P:(tt + 1) * P],
                                rhs=wT[:, kt, oc * 512:(oc + 1) * 512],
                                start=(kt == 0), stop=(kt == KT - 1))
                        sink(tt, oc, ps)

            def spill_sink(dst):
                def _sink(tt, oc, ps):
                    sb = cast_p.tile([P, 512], bf16, tag="spill")
                    nc.vector.tensor_copy(out=sb, in_=ps)
                    nc.sync.dma_start(
                        out=dst[tt * P:(tt + 1) * P, oc * 512:(oc + 1) * 512],
                        in_=sb)
                return _sink

            with tc.tile_pool(name="xT_pool", bufs=1) as xT_pool:
                xT = xT_pool.tile([P, KT, T], bf16)      # x.T  (i on part)
                load_transposed(x_d, xT)
                # q has SCALE folded in host-side (Wq pre-scaled)
                project(xT, wq_d, spill_sink(q_spill))
                project(xT, wk_d, spill_sink(k_spill))
                project(xT, wv_d, spill_sink(v_spill))

            # Wo transposed, resident for the output projection
            woT = wpool.tile([P, KT, DIM], bf16, tag="wT")
            load_transposed(wo_d, woT)

            # ---- attention per 128-token tile ----
            inv2d = 1.0 / (2.0 * D)
            one_m_lam_init = 1.0 - LAM_INIT

            with (
                tc.tile_pool(name="attn2", bufs=2) as attn_p,
                tc.tile_pool(name="attn1", bufs=1) as attn_s,
            ):
                for tt in range(NT):
                    q_t = attn_p.tile([P, DIM], bf16, tag="q_t")
                    k_t = attn_p.tile([P, DIM], bf16, tag="k_t")
                    v_t = attn_p.tile([P, DIM], bf16, tag="v_t")
                    nc.sync.dma_start(
                        out=q_t, in_=q_spill[tt * P:(tt + 1) * P, :])
                    nc.sync.dma_start(
                        out=k_t, in_=k_spill[tt * P:(tt + 1) * P, :])
                    nc.sync.dma_start(
                        out=v_t, in_=v_spill[tt * P:(tt + 1) * P, :])

                    # deinterleaved views: [p, qi(2), h(16), d(64)]
                    q_v = q_t.rearrange("p (h d q) -> p q h d", d=D, q=2)
                    k_v = k_t.rearrange("p (h d q) -> p q h d", d=D, q=2)

                    # s[t, i, j] = sum_d q[t,i,:] k[t,j,:]  (i,j in [0,32))
                    s_all = attn_s.tile([P, 32, 32], f32, tag="s_all")
                    for j in range(32):
                        qi_j, h_j = divmod(j, H)
                        eng = nc.gpsimd if (j % 3 == 2) else nc.vector
                        prod = attn_p.tile([P, 2, H, D], bf16, tag="prod")
                        kb = k_v[:, qi_j:qi_j + 1, h_j:h_j + 1, :]
                        eng.tensor_tensor(
                            out=prod, in0=q_v,
                            in1=kb.to_broadcast([P, 2, H, D]), op=Alu.mult)
                        nc.vector.tensor_reduce(
                            out=s_all[:, :, j], in_=prod,
                            axis=Axis.X, op=Alu.add)

                    # s layout [p, i, j]; softmax over j; exp in place
                    nc.scalar.activation(
                        out=s_all.rearrange("p a b -> p (a b)"),
                        in_=s_all.rearrange("p a b -> p (a b)"), func=Act.Exp)

                    z = attn_s.tile([P, 32], f32, tag="z")
                    nc.vector.tensor_reduce(
                        out=z, in_=s_all, axis=Axis.X, op=Alu.add)
                    rz = attn_s.tile([P, 32], f32, tag="rz")
                    nc.vector.reciprocal(out=rz, in_=z)

                    # P[a,h] = e[a,h]*rz[a] - lam * e[16+a,16+h]*rz[16+a]
                    p1 = attn_s.tile([P, H, H], f32, tag="p1")
                    nc.vector.tensor_tensor(
                        out=p1, in0=s_all[:, 0:H, 0:H],
                        in1=rz[:, 0:H, None].to_broadcast([P, H, H]),
                        op=Alu.mult)
                    p2 = attn_s.tile([P, H, H], f32, tag="p2")
                    nc.vector.tensor_tensor(
                        out=p2, in0=s_all[:, H:32, H:32],
                        in1=rz[:, H:32, None].to_broadcast([P, H, H]),
                        op=Alu.mult)
                    pm = attn_s.tile([P, H, H], bf16, tag="pm")
                    nc.vector.scalar_tensor_tensor(
                        out=pm, in0=p2, scalar=-lam, in1=p1,
                        op0=Alu.mult, op1=Alu.add)

                    # u[t, a, e] = sum_h P[a,h] v[t, h, e]
                    v_r = v_t.rearrange("p (h e) -> p e h", h=H)  # [p,128,16]
                    u = attn_s.tile([P, H, 2 * D], f32, tag="u")
                    for ah in range(8):       # a in chunks of 2
                        wp = attn_p.tile([P, 2, 2 * D, H], bf16, tag="wprod")
                        eng = nc.gpsimd if (ah % 3 == 2) else nc.vector
                        eng.tensor_tensor(
                            out=wp,
                            in0=pm[:, 2 * ah:2 * ah + 2, None, :]
                                .to_broadcast([P, 2, 2 * D, H]),
                            in1=v_r[:, None, :, :]
                                .to_broadcast([P, 2, 2 * D, H]),
                            op=Alu.mult)
                        nc.vector.tensor_reduce(
                            out=u[:, 2 * ah:2 * ah + 2, :], in_=wp,
                            axis=Axis.X, op=Alu.add)

                    # RMS norm over e (2D=128) then * (1-LAM_INIT)
                    usq = attn_s.tile([P, H, 2 * D], bf16, tag="usq")
                    nc.scalar.activation(
                        out=usq.rearrange("p a e -> p (a e)"),
                        in_=u.rearrange("p a e -> p (a e)"), func=Act.Square)
                    m2 = attn_s.tile([P, H], f32, tag="m2")
                    nc.vector.tensor_reduce(
                        out=m2, in_=usq, axis=Axis.X, op=Alu.add)
                    sd = attn_s.tile([P, H], f32, tag="sd")
                    nc.scalar.activation(
                        out=sd, in_=m2, func=Act.Sqrt,
                        bias=eps_sb, scale=inv2d)
                    rstd = attn_s.tile([P, H], f32, tag="rstd")
                    nc.vector.reciprocal(out=rstd, in_=sd)

                    on_t = attn_s.tile([P, H, 2 * D], bf16, tag="on_t")
                    nc.vector.scalar_tensor_tensor(
                        out=on_t, in0=u, scalar=one_m_lam_init,
                        in1=rstd[:, :, None].to_broadcast([P, H, 2 * D]),
                        op0=Alu.mult, op1=Alu.mult)

                    # transpose out_n tile -> [feat, t] for Wo projection
                    onT = attn_s.tile([P, KT, P], bf16, tag="onT")
                    on_flat = on_t.rearrange("p a e -> p (a e)")
                    for c in range(KT):
                        ps = psum_tr.tile([P, P], bf16, tag="trps")
                        nc.tensor.transpose(
                            ps, on_flat[:, c * P:(c + 1) * P], ident)
                        nc.vector.tensor_copy(out=onT[:, c, :], in_=ps)

                    # y[tt] = out_n @ Wo.T + bo
                    y_sb = attn_p.tile([P, DIM], f32, tag="y_sb")
                    for oc in range(OC):
                        ps = psum_mm.tile([P, 512], f32, tag="mmps")
                        for kt in range(KT):
                            nc.tensor.matmul(
                                ps, lhsT=onT[:, kt, :],
                                rhs=woT[:, kt, oc * 512:(oc + 1) * 512],
                                start=(kt == 0), stop=(kt == KT - 1))
                        nc.vector.tensor_add(
                            out=y_sb[:, oc * 512:(oc + 1) * 512], in0=ps,
                            in1=bo_sb[:, oc * 512:(oc + 1) * 512])
                    nc.sync.dma_start(
                        out=y_d[tt * P:(tt + 1) * P, :], in_=y_sb)

    nc.finalize()
    return nc


def kernel(**inputs):
    x = np.asarray(inputs["x"], dtype=np.float32)
    Wq = np.asarray(inputs["Wq"], dtype=np.float32)
    Wk = np.asarray(inputs["Wk"], dtype=np.float32)
    Wv = np.asarray(inputs["Wv"], dtype=np.float32)
    Wo = np.asarray(inputs["Wo"], dtype=np.float32)
    bo = np.asarray(inputs["bo"], dtype=np.float32)
    lq1 = np.asarray(inputs["lq1"], dtype=np.float32)
    lq2 = np.asarray(inputs["lq2"], dtype=np.float32)
    lk1 = np.asarray(inputs["lk1"], dtype=np.float32)
    lk2 = np.asarray(inputs["lk2"], dtype=np.float32)

    lam = float(np.exp(np.sum(lq1 * lk1)) - np.exp(np.sum(lq2 * lk2))
                + LAM_INIT)

    b, n, _ = x.shape
    xt = np.ascontiguousarray(x.reshape(b * n, DIM))
    wq_s = np.ascontiguousarray(Wq * np.float32(SCALE))
    bo_rep = np.ascontiguousarray(np.broadcast_to(bo, (P, DIM)))

    key = round(lam, 6)
    if key not in _PROGRAM_CACHE:
        _PROGRAM_CACHE[key] = _build(lam)
    nc = _PROGRAM_CACHE[key]

    from concourse.bass_utils import run_bass_kernel_spmd

    in_maps = []
    for c in range(NCORES):
        in_maps.append({
            "x": np.ascontiguousarray(xt[c * T:(c + 1) * T]),
            "Wq": wq_s,
            "Wk": Wk,
            "Wv": Wv,
            "Wo": Wo,
            "bo": bo_rep,
        })

    res = run_bass_kernel_spmd(nc, in_maps, core_ids=list(range(NCORES)))
    globals()["LAST_RESULT"] = res
    y = np.concatenate([res.results[c]["y"] for c in range(NCORES)], axis=0)
    return y.reshape(b, n, DIM).astype(np.float32)



# revision 16
# speedup vs baseline: 1.5106x; 1.5106x over previous
"""Differential attention kernel for 8 TRN2 NeuronCores (v3).

Data-parallel over 16384 tokens (2048/core).  Host pre-transposes and
feature-permutes the weights (bf16) so every DVE access is packed-
contiguous; x is pre-transposed per core.  Token sweep runs in two
halves so the DVE-bound attention (gram + softmax + diff@v + RMS)
overlaps the PE projection phases of the other half.  Reductions use
bf16 pairwise trees (2x DVE mode) instead of tensor_reduce (1x).
Weights rotate through two 64KB SBUF slots; Wo stays resident.
"""

import numpy as np

DIM = 2048
H = 16
D = 64
LAYER_NUM = 12
LAM_INIT = 0.8 - 0.6 * float(np.exp(-0.3 * LAYER_NUM))
SCALE = D ** -0.5
EPS = 1e-5

NCORES = 8
T = 2048                  # tokens per core
NT = T // 128             # 16 token tiles per core
NH = NT // 2              # tiles per half
P = 128
OC = 4                    # psum chunks of 512 per projection
KT = DIM // P             # 16 contraction tiles

_PROGRAM_CACHE = {}


def _build(lam: float):
    import concourse.bass as bass
    import concourse.tile as tile
    from concourse import bacc
    from concourse import mybir
    from concourse.masks import make_identity

    f32 = mybir.dt.float32
    bf16 = mybir.dt.bfloat16
    Alu = mybir.AluOpType
    Act = mybir.ActivationFunctionType
    Axis = mybir.AxisListType

    nc = bacc.Bacc("TRN2", target_bir_lowering=False)

    xT_d = nc.dram_tensor("xT", [DIM, T], bf16, kind="ExternalInput")
    wq_d = nc.dram_tensor("WqT", [DIM, DIM], bf16, kind="ExternalInput")
    wk_d = nc.dram_tensor("WkT", [DIM, DIM], bf16, kind="ExternalInput")
    wv_d = nc.dram_tensor("WvT", [DIM, DIM], bf16, kind="ExternalInput")
    wo_d = nc.dram_tensor("WoT", [DIM, DIM], bf16, kind="ExternalInput")
    bo_d = nc.dram_tensor("bo", [1, DIM], bf16, kind="ExternalInput")
    y_d = nc.dram_tensor("y", [T, DIM], bf16, kind="ExternalOutput")

    xT_v = xT_d.rearrange("(kt p) t -> p kt t", p=P)

    def w_view(w):
        return w.rearrange("(kt p) o -> p kt o", p=P)

    inv2d = 1.0 / (2.0 * D)
    one_m_lam_init = 1.0 - LAM_INIT

    with tile.TileContext(nc) as tc, \
            nc.allow_low_precision(reason="bf16 intermediates validated"):
        with (
            tc.tile_pool(name="persist", bufs=1) as persist,
            tc.tile_pool(name="wslots", bufs=1) as wslots,
            tc.tile_pool(name="xstream", bufs=2) as xpool,
            tc.tile_pool(name="spillc", bufs=3) as spill_p,
            tc.tile_pool(name="qk", bufs=2) as qk_p,
            tc.tile_pool(name="vtile", bufs=2) as v_p,
            tc.tile_pool(name="gbuf", bufs=1) as g_p,
            tc.tile_pool(name="wbuf", bufs=1) as w_p,
            tc.tile_pool(name="attn", bufs=1) as spool,
            tc.tile_pool(name="upool", bufs=1) as upool,
            tc.tile_pool(name="ont", bufs=1) as on_p,
            tc.tile_pool(name="onTp", bufs=1) as onT_p,
            tc.tile_pool(name="psum_mm", bufs=4, space="PSUM") as psum_mm,
            tc.tile_pool(name="psum_y", bufs=2, space="PSUM") as psum_y,
            tc.tile_pool(name="psum_tr", bufs=2, space="PSUM") as psum_tr,
            tc.tile_pool(name="dram", bufs=1, space="DRAM") as dram,
        ):
            ident = persist.tile([P, P], bf16)
            make_identity(nc, ident)
            eps_sb = persist.tile([P, 1], f32)
            nc.vector.memset(eps_sb, EPS)
            ones_sb = persist.tile([1, P], bf16)
            nc.vector.memset(ones_sb, 1.0)
            bo_sb = persist.tile([1, DIM], bf16)
            nc.sync.dma_start(out=bo_sb, in_=bo_d[:, :])

            slot0 = wslots.tile([P, KT, DIM], bf16, tag="s0")
            slot1 = wslots.tile([P, KT, DIM], bf16, tag="s1")

            q_spill = dram.tile([T, DIM], bf16)
            k_spill = dram.tile([T, DIM], bf16)

            def proj_tile(tt, wslot, sink):
                xt = xpool.tile([P, KT, P], bf16, tag="xt")
                nc.sync.dma_start(
                    out=xt, in_=xT_v[:, :, tt * P:(tt + 1) * P])
                pss = [psum_mm.tile([P, 512], f32, tag="mm", name=f"mm{oc}")
                       for oc in range(OC)]
                for kt in range(KT):
                    for oc in range(OC):
                        nc.tensor.matmul(
                            pss[oc],
                            lhsT=xt[:, kt, :],
                            rhs=wslot[:, kt, oc * 512:(oc + 1) * 512],
                            start=(kt == 0), stop=(kt == KT - 1))
                for oc in range(OC):
                    sink(tt, oc, pss[oc])

            def spill_sink(dst):
                def _sink(tt, oc, ps):
                    sb = spill_p.tile([P, 512], bf16, tag="sp")
                    nc.scalar.copy(out=sb, in_=ps)
                    nc.sync.dma_start(
                        out=dst[tt * P:(tt + 1) * P,
                                oc * 512:(oc + 1) * 512],
                        in_=sb)
                return _sink

            def v_proj(tt, wslot):
                v_t = v_p.tile([P, DIM], bf16, tag="v")

                def _sink(_tt, oc, ps):
                    nc.scalar.copy(
                        out=v_t[:, oc * 512:(oc + 1) * 512], in_=ps)
                proj_tile(tt, wslot, _sink)
                return v_t

            # pairwise-halving tree sum over the innermost axis of a
            # [P, a, b, n] buffer (in place), final level -> final_out
            def tree_sum(buf, n_inner, final_out, eng):
                half = n_inner // 2
                while half > 1:
                    eng.tensor_tensor(
                        out=buf[:, :, :, 0:half],
                        in0=buf[:, :, :, 0:half],
                        in1=buf[:, :, :, half:2 * half], op=Alu.add)
                    half //= 2
                eng.tensor_tensor(
                    out=final_out, in0=buf[:, :, :, 0],
                    in1=buf[:, :, :, 1], op=Alu.add)

            def attn_vector(tt, v_t):
                q_t = qk_p.tile([P, DIM], bf16, tag="q")
                k_t = qk_p.tile([P, DIM], bf16, tag="k")
                nc.sync.dma_start(
                    out=q_t, in_=q_spill[tt * P:(tt + 1) * P, :])
                nc.sync.dma_start(
                    out=k_t, in_=k_spill[tt * P:(tt + 1) * P, :])
                q_v = q_t.rearrange("p (q h d) -> p q h d", q=2, h=H)
                k_v = k_t.rearrange("p (q h d) -> p q h d", q=2, h=H)
                q_f = q_t.rearrange("p (i d) -> p i d", d=D)

                # s[p, j, i] = sum_d q[p,i,:]*k[p,j,:]
                s_all = spool.tile([P, 32, 32], bf16, tag="s")
                for g in range(16):         # groups of 2 j
                    gb = g_p.tile([P, 2, 32, D], bf16, tag="gb")
                    for jj in range(2):
                        j = 2 * g + jj
                        qi_j, h_j = divmod(j, H)
                        eng = nc.gpsimd if (j % 8 < 3) else nc.vector
                        eng.tensor_tensor(
                            out=gb[:, jj], in0=q_f,
                            in1=k_v[:, qi_j:qi_j + 1, h_j, :]
                                .to_broadcast([P, 32, D]),
                            op=Alu.mult)
                    tree_sum(gb, D, s_all[:, 2 * g:2 * g + 2, :],
                             nc.vector)

                s_flat = s_all.rearrange("p a b -> p (a b)")
                nc.scalar.activation(out=s_flat, in_=s_flat, func=Act.Exp)

                # z[i] = sum_j e[j,i]  (tree over j)
                zt = spool.tile([P, 16, 32], bf16, tag="zt")
                nc.vector.tensor_tensor(
                    out=zt, in0=s_all[:, 0:16, :], in1=s_all[:, 16:32, :],
                    op=Alu.add)
                for half in (8, 4, 2):
                    nc.vector.tensor_tensor(
                        out=zt[:, 0:half, :], in0=zt[:, 0:half, :],
                        in1=zt[:, half:2 * half, :], op=Alu.add)
                z = spool.tile([P, 32], f32, tag="z")
                nc.vector.tensor_tensor(
                    out=z, in0=zt[:, 0, :], in1=zt[:, 1, :], op=Alu.add)
                rz = spool.tile([P, 32], f32, tag="rz")
                nc.vector.reciprocal(out=rz, in_=z)

                # pm[a, h] = e1[h,a]*rz[a] - lam*e2[h,a]*rz[16+a]
                s_T = s_all.rearrange("p j i -> p i j")
                p1 = spool.tile([P, H, H], f32, tag="p1")
                nc.vector.tensor_tensor(
                    out=p1, in0=s_T[:, 0:H, 0:H],
                    in1=rz[:, 0:H, None].to_broadcast([P, H, H]),
                    op=Alu.mult)
                p2 = spool.tile([P, H, H], f32, tag="p2")
                nc.vector.tensor_tensor(
                    out=p2, in0=s_T[:, H:32, H:32],
                    in1=rz[:, H:32, None].to_broadcast([P, H, H]),
                    op=Alu.mult)
                pm = spool.tile([P, H, H], bf16, tag="pm")
                nc.vector.scalar_tensor_tensor(
                    out=pm, in0=p2, scalar=-lam, in1=p1,
                    op0=Alu.mult, op1=Alu.add)

                # u[a, e] = sum_h pm[a,h] v[e*H+h]
                v_r = v_t.rearrange("p (e h) -> p e h", h=H)
                u = upool.tile([P, H, 2 * D], bf16, tag="u")
                for ah in range(8):
                    eng = nc.gpsimd if (ah in (2, 6)) else nc.vector
                    wp = w_p.tile([P, 2, 2 * D, H], bf16, tag="wp")
                    eng.tensor_tensor(
                        out=wp,
                        in0=pm[:, 2 * ah:2 * ah + 2, None, :]
                            .to_broadcast([P, 2, 2 * D, H]),
                        in1=v_r[:, None, :, :]
                            .to_broadcast([P, 2, 2 * D, H]),
                        op=Alu.mult)
                    tree_sum(wp, H, u[:, 2 * ah:2 * ah + 2, :],
                             nc.vector)

                # RMS over e then * (1-LAM_INIT)
                usq = upool.tile([P, H, 2 * D], bf16, tag="usq")
                nc.scalar.activation(
                    out=usq.rearrange("p a e -> p (a e)"),
                    in_=u.rearrange("p a e -> p (a e)"),
                    func=Act.Square)
                m2 = spool.tile([P, H], f32, tag="m2")
                nc.vector.tensor_reduce(
                    out=m2, in_=usq, axis=Axis.X, op=Alu.add)
                sd = spool.tile([P, H], f32, tag="sd")
                nc.scalar.activation(
                    out=sd, in_=m2, func=Act.Sqrt,
                    bias=eps_sb, scale=inv2d)
                rstd = spool.tile([P, H], f32, tag="rstd")
                nc.vector.reciprocal(out=rstd, in_=sd)

                on_t = on_p.tile([P, H, 2 * D], bf16, tag="on")
                nc.vector.scalar_tensor_tensor(
                    out=on_t, in0=u, scalar=one_m_lam_init,
                    in1=rstd[:, :, None].to_broadcast([P, H, 2 * D]),
                    op0=Alu.mult, op1=Alu.mult)
                return on_t

            def attn_pe(tt, on_t):
                onT = onT_p.tile([P, KT, P], bf16, tag="onT")
                on_flat = on_t.rearrange("p a e -> p (a e)")
                for c in range(KT):
                    pst = psum_tr.tile([P, P], bf16, tag="tr")
                    nc.tensor.transpose(
                        pst, on_flat[:, c * P:(c + 1) * P], ident)
                    nc.scalar.copy(out=onT[:, c, :], in_=pst)
                for oc in range(OC):
                    ps = psum_y.tile([P, 512], f32, tag="y")
                    nc.tensor.matmul(
                        ps, lhsT=ones_sb,
                        rhs=bo_sb[:, oc * 512:(oc + 1) * 512],
                        start=True, stop=False)
                    for kt in range(KT):
                        nc.tensor.matmul(
                            ps, lhsT=onT[:, kt, :],
                            rhs=slot1[:, kt, oc * 512:(oc + 1) * 512],
                            start=False, stop=(kt == KT - 1))
                    y_sb = spill_p.tile([P, 512], bf16, tag="sp")
                    nc.scalar.copy(out=y_sb, in_=ps)
                    nc.sync.dma_start(
                        out=y_d[tt * P:(tt + 1) * P,
                                oc * 512:(oc + 1) * 512],
                        in_=y_sb)

            # ---------------- program ----------------
            # slot0: wq_A -> wv_A -> wq_B -> wk_B -> wv_B
            # slot1: wk_A -> wo (resident for both halves' output proj)
            nc.sync.dma_start(out=slot0, in_=w_view(wq_d))
            nc.sync.dma_start(out=slot1, in_=w_view(wk_d))

            pending = None
            for half in range(2):
                t0 = half * NH
                for t in range(t0, t0 + NH):       # Q phase (slot0)
                    proj_tile(t, slot0, spill_sink(q_spill))
                    if pending is not None:
                        attn_pe(*pending)
                        pending = None
                if half == 0:
                    nc.sync.dma_start(out=slot0, in_=w_view(wv_d))
                    for t in range(t0, t0 + NH):   # K phase (slot1)
                        proj_tile(t, slot1, spill_sink(k_spill))
                    nc.sync.dma_start(out=slot1, in_=w_view(wo_d))
                else:
                    nc.sync.dma_start(out=slot0, in_=w_view(wk_d))
                    for t in range(t0, t0 + NH):   # K phase (slot0)
                        proj_tile(t, slot0, spill_sink(k_spill))
                    nc.sync.dma_start(out=slot0, in_=w_view(wv_d))
                for t in range(t0, t0 + NH):       # V + attention
                    v_t = v_proj(t, slot0)
                    on_t = attn_vector(t, v_t)
                    if pending is not None:
                        attn_pe(*pending)
                    pending = (t, on_t)
                if half == 0:
                    nc.sync.dma_start(out=slot0, in_=w_view(wq_d))
            attn_pe(*pending)

    nc.finalize()
    return nc


def kernel(**inputs):
    import ml_dtypes
    bfd = ml_dtypes.bfloat16

    x = np.asarray(inputs["x"], dtype=np.float32)
    Wq = np.asarray(inputs["Wq"], dtype=np.float32)
    Wk = np.asarray(inputs["Wk"], dtype=np.float32)
    Wv = np.asarray(inputs["Wv"], dtype=np.float32)
    Wo = np.asarray(inputs["Wo"], dtype=np.float32)
    bo = np.asarray(inputs["bo"], dtype=np.float32)
    lq1 = np.asarray(inputs["lq1"], dtype=np.float32)
    lq2 = np.asarray(inputs["lq2"], dtype=np.float32)
    lk1 = np.asarray(inputs["lk1"], dtype=np.float32)
    lk2 = np.asarray(inputs["lk2"], dtype=np.float32)

    lam = float(np.exp(np.sum(lq1 * lk1)) - np.exp(np.sum(lq2 * lk2))
                + LAM_INIT)

    b, n, _ = x.shape
    xt = x.reshape(b * n, DIM)

    # feature permutations: q/k out-features in (qi, h, d) order,
    # v out-features in (e, h) order
    nf = np.arange(DIM)
    of_qk = ((nf // D) % H) * (2 * D) + (nf % D) * 2 + nf // (H * D)
    of_v = (nf % H) * (2 * D) + nf // H

    wqT = np.ascontiguousarray((Wq * np.float32(SCALE))[of_qk, :].T
                               ).astype(bfd)
    wkT = np.ascontiguousarray(Wk[of_qk, :].T).astype(bfd)
    wvT = np.ascontiguousarray(Wv[of_v, :].T).astype(bfd)
    woT = np.ascontiguousarray(Wo.T).astype(bfd)
    boB = bo.reshape(1, DIM).astype(bfd)

    key = round(lam, 6)
    if key not in _PROGRAM_CACHE:
        _PROGRAM_CACHE[key] = _build(lam)
    nc = _PROGRAM_CACHE[key]

    from concourse.bass_utils import run_bass_kernel_spmd

    in_maps = []
    for c in range(NCORES):
        xTc = np.ascontiguousarray(xt[c * T:(c + 1) * T].T).astype(bfd)
        in_maps.append({
            "xT": xTc,
            "WqT": wqT,
            "WkT": wkT,
            "WvT": wvT,
            "WoT": woT,
            "bo": boB,
        })

    res = run_bass_kernel_spmd(nc, in_maps, core_ids=list(range(NCORES)))
    globals()["LAST_RESULT"] = res
    y = np.concatenate(
        [np.asarray(res.results[c]["y"]) for c in range(NCORES)], axis=0)
    return y.reshape(b, n, DIM).astype(np.float32)
